# revision 1
# baseline (speedup 1.0000x reference)
"""Causal single-head attention (B=4, S=4096, D=2048, d_att=128) on 8 TRN2 cores.

Strategy (all shapes hardcoded; self-contained):
  Phase 1 (SPMD x8): fused QKV projection, data-parallel over the 16384
    flattened (b, s) rows -- 2048 rows per core. Each core computes
    Q^T/K^T/V^T [128, 2048] for its rows in bf16.
  Host: reshuffles Q/K/V into 5 balanced (q-block, k-block) "pairs" per core.
    Core (b, 0) handles query blocks {0, 3} of batch b, core (b, 1) blocks
    {1, 2} (blocks of 1024). Expanding each causal row-block into its
    k-prefix gives exactly 5 block-pairs per core: [diag, full, full, full,
    diag], identical structure on every core -> one SPMD program.
  Phase 2 (SPMD x8): per pair, scores S^T = K Q^T (PE), E = exp(scale*S^T)
    (ACT, unnormalized -- max|score| ~ 9, so no row-max pass is needed in
    fp32/bf16 range), causal triangle zeroing of the diagonal 128-block via
    gpsimd affine_select, then O^T = V^T E^T on PE, plus the softmax
    denominator l = 1^T E^T on PE over a 2-level DVE pre-sum tree of E (4x
    fewer PE columns). Diagonal pairs skip fully-masked 128-wide column
    strips exactly. Partials (acc, l) are combined on the host:
    out = sum(acc) / sum(l) per query row.
  Compute dtype bf16 (TensorE fp32 is 4 cycles/row vs bf16's 1), fp32 PSUM
  accumulation; end-to-end max-rel-err vs the fp32 reference ~ 5.7e-3.
"""

import numpy as np
import ml_dtypes
import orjson

import concourse.bass as bass
import concourse.tile as tile
import concourse.mybir as mybir
from concourse.bass_utils import run_bass_kernel_spmd

BF16 = ml_dtypes.bfloat16
N_CORES = 8
B, S, D, DA = 4, 4096, 2048, 128
ROWS = B * S  # 16384
RPC = ROWS // N_CORES  # 2048 rows per core in phase 1
QBLK = 1024  # query/key block size for pair decomposition
NKT2 = QBLK // 128  # key tiles per pair
NPAIR = 5
SCALE = 1.0 / np.sqrt(DA)

# (q_block, k_block) pairs per half; p0 and p4 are causal-diagonal on all cores
PAIRS_H0 = [(0, 0), (3, 0), (3, 1), (3, 2), (3, 3)]
PAIRS_H1 = [(1, 1), (1, 0), (2, 0), (2, 1), (2, 2)]
DIAG_PAIRS = (0, 4)


# ---------------------------------------------------------------------------
# Walrus workaround: this neuronxcc build rejects >1 sync-wait command per
# instruction ("Too many sync wait commands"). Excess on_wait entries are
# hoisted onto preceding same-engine NoOps -- semantically identical since
# each engine executes its queue in order.
# ---------------------------------------------------------------------------
def _fix_bir_json(bir: bytes, max_waits: int = 1) -> bytes:
    m = orjson.loads(bir)
    counter = [0]
    for fn in m.get("functions", []):
        for bb in fn.get("blocks", []):
            out = []
            for inst in bb.get("instructions", []):
                si = inst.get("sync_info")
                waits = (si or {}).get("on_wait") or []
                if len(waits) > max_waits:
                    excess, keep = waits[:-max_waits], waits[-max_waits:]
                    for i in range(0, len(excess), max_waits):
                        counter[0] += 1
                        out.append(
                            {
                                "engine": inst["engine"],
                                "ins": [],
                                "name": f"I-waitfix-{counter[0]}",
                                "opcode": "NoOp",
                                "outs": [],
                                "sync_info": {
                                    "on_update": [],
                                    "on_wait": excess[i : i + max_waits],
                                },
                            }
                        )
                    si["on_wait"] = keep
                out.append(inst)
            bb["instructions"] = out
    return orjson.dumps(m)


def _patch_bass(nc):
    orig = nc.to_json_bytes
    nc.to_json_bytes = lambda: _fix_bir_json(orig())
    return nc


# ---------------------------------------------------------------------------
# Phase 1: QKV projection. Per core: xT [D, RPC] bf16, wcat [128, 16*384]
# bf16 (d-tile-major repack of W_qkv^T) -> qT/kT/vT [DA, RPC] bf16.
# ---------------------------------------------------------------------------
NDT = D // 128  # 16 contraction tiles
CH1 = 256  # phase-1 moving-chunk width
NCH = RPC // CH1


def build_qkv_nc():
    nc = bass.Bass(
        "TRN2", target_bir_lowering=False, debug=False, enable_partition_id=False
    )
    bf = mybir.dt.bfloat16
    xT = nc.dram_tensor("xT", [D, RPC], bf, kind="ExternalInput").ap()
    # o-major repack: wcat[p, o*D + i*128 + e] = W_qkv[o*128+e, i*128+p]
    wcat = nc.dram_tensor("wcat", [128, 3 * D], bf, kind="ExternalInput").ap()
    outs = [
        nc.dram_tensor(f"{n}T", [DA, RPC], bf, kind="ExternalOutput").ap()
        for n in "qkv"
    ]

    with tile.TileContext(nc) as tc:
        import contextlib

        with contextlib.ExitStack() as ctx:
            xpool = ctx.enter_context(tc.tile_pool(name="xp", bufs=1))
            wpool = ctx.enter_context(tc.tile_pool(name="wp", bufs=1))
            opool = ctx.enter_context(tc.tile_pool(name="op", bufs=8))
            pspool = ctx.enter_context(tc.tile_pool(name="ps", bufs=4, space="PSUM"))

            # Warm up the PE (HAM clock ramp) on dummy data during the
            # initial DMA window so the real matmuls start at full clock.
            dummy = wpool.tile([128, 512], bf, tag="dummy")
            nc.vector.memset(dummy[:], 1.0)
            wps = pspool.tile([128, CH1], mybir.dt.float32, tag="ps", name="warm")
            for _ in range(14):
                nc.tensor.matmul(
                    wps[:],
                    lhsT=dummy[:, 0:128],
                    rhs=dummy[:, 0:CH1],
                    start=True,
                    stop=True,
                )

            # W arrives as three per-output pieces so o=0 compute can start
            # after just wq + the first x chunk
            wsb = wpool.tile([128, 3 * D], bf)
            nc.sync.dma_start(wsb[:, 0:D], wcat[:, 0:D])

            def load_xc(c):
                sl = slice(c * CH1, (c + 1) * CH1)
                xc = xpool.tile([128, NDT, CH1], bf, tag=f"xc{c}", name=f"xc{c}")
                nc.sync.dma_start(
                    xc[:], xT[:, sl].rearrange("(i p) r -> p i r", p=128)
                )
                return xc

            xcs = [load_xc(0)]
            nc.sync.dma_start(wsb[:, D : 2 * D], wcat[:, D : 2 * D])
            nc.sync.dma_start(wsb[:, 2 * D :], wcat[:, 2 * D :])
            xcs.append(load_xc(1))

            for c in range(NCH):
                if c + 2 < NCH:
                    xcs.append(load_xc(c + 2))
                sl = slice(c * CH1, (c + 1) * CH1)
                for o in range(3):
                    ps = pspool.tile([128, CH1], mybir.dt.float32, tag="ps")
                    for i in range(NDT):
                        nc.tensor.matmul(
                            ps[:],
                            lhsT=wsb[:, o * D + i * 128 : o * D + (i + 1) * 128],
                            rhs=xcs[c][:, i, :],
                            start=(i == 0),
                            stop=(i == NDT - 1),
                        )
                    ob = opool.tile([128, CH1], bf, tag="ob")
                    nc.scalar.copy(ob[:], ps[:])
                    nc.sync.dma_start(outs[o][:, sl], ob[:])
    return _patch_bass(nc)


# ---------------------------------------------------------------------------
# Phase 2: pair attention. Per core:
#   qT, kT [NPAIR, DA, QBLK] bf16; v [NPAIR, 128, QBLK] bf16 where
#   v[p, kk, kt*128 + d] = V[kt*128 + kk, d] (host pre-permuted so the DMA is
#   contiguous and v[:, :, kt*128:(kt+1)*128] is the [k, d] tile for AV lhsT)
#   -> accT [NPAIR, DA, QBLK] f32 (unnormalized O^T), lsum [NPAIR, QBLK] f32
# ---------------------------------------------------------------------------
def build_attn_nc():
    nc = bass.Bass(
        "TRN2", target_bir_lowering=False, debug=False, enable_partition_id=False
    )
    bf = mybir.dt.bfloat16
    f32 = mybir.dt.float32
    # one fused input: qkv[p, 0] = Q^T block, [p, 1] = K^T block,
    # [p, 2] = V block pre-permuted ([kk, kt*128+d] = V[kt*128+kk, d])
    qkv = nc.dram_tensor("qkv", [NPAIR, 3, 128, QBLK], bf, kind="ExternalInput").ap()
    accT = nc.dram_tensor("accT", [NPAIR, DA, QBLK], f32, kind="ExternalOutput").ap()
    lsum = nc.dram_tensor("lsum", [NPAIR, QBLK], f32, kind="ExternalOutput").ap()

    NKT = QBLK // 128  # 8 key tiles per pair
    Exp = mybir.ActivationFunctionType.Exp

    with tile.TileContext(nc) as tc:
        import contextlib

        with contextlib.ExitStack() as ctx:
            const = ctx.enter_context(tc.tile_pool(name="const", bufs=1))
            inp = ctx.enter_context(tc.tile_pool(name="inp", bufs=2))
            epool = ctx.enter_context(tc.tile_pool(name="ep", bufs=2))
            opool = ctx.enter_context(tc.tile_pool(name="op", bufs=3))
            ps_s = ctx.enter_context(tc.tile_pool(name="pss", bufs=1, space="PSUM"))
            ps_o = ctx.enter_context(tc.tile_pool(name="pso", bufs=1, space="PSUM"))
            ps_l = ctx.enter_context(tc.tile_pool(name="psl", bufs=1, space="PSUM"))

            # ones column for l matmuls
            ones = const.tile([128, 1], bf)
            nc.vector.memset(ones[:], 1.0)
            # PE warm-up on dummy data during the first input DMA
            dummy = const.tile([128, 512], bf)
            nc.vector.memset(dummy[:], 1.0)
            wps = ps_o.tile([128, 512], f32, tag="po", name="warm")
            for _ in range(8):
                nc.tensor.matmul(
                    wps[:],
                    lhsT=dummy[:, 0:128],
                    rhs=dummy[:],
                    start=True,
                    stop=True,
                )

            for p in range(NPAIR):
                diag = p in DIAG_PAIRS
                qkv_s = inp.tile([128, 3, QBLK], bf, tag="qkv")
                # q+k first (gates the scores), v separately (needed later)
                nc.sync.dma_start(
                    qkv_s[:, 0:2, :], qkv[p, 0:2].rearrange("t p q -> p t q")
                )
                nc.sync.dma_start(qkv_s[:, 2, :], qkv[p, 2])
                qt_s = qkv_s[:, 0, :]
                kt_s = qkv_s[:, 1, :]
                v_s = qkv_s[:, 2, :]

                et, es1, es = [], [], []

                def emit_kt(kt, diag=diag, qt_s=qt_s, kt_s=kt_s, et=et):
                    q0 = 128 * kt if diag else 0  # causal: keys kt*128.. only
                    pss = ps_s.tile(
                        [128, QBLK], f32, tag=f"pss{kt % 3}", name=f"pss{kt % 3}"
                    )
                    c0 = q0
                    while c0 < QBLK:
                        c1 = min((c0 // 512 + 1) * 512, QBLK)
                        nc.tensor.matmul(
                            pss[:, c0:c1],
                            lhsT=kt_s[:, kt * 128 : (kt + 1) * 128],
                            rhs=qt_s[:, c0:c1],
                            start=True,
                            stop=True,
                        )
                        c0 = c1
                    e = epool.tile([128, QBLK], bf, tag=f"et{kt}", name=f"et{kt}")
                    nc.scalar.activation(
                        e[:, q0:QBLK], pss[:, q0:QBLK], Exp, scale=SCALE
                    )
                    if diag:
                        # triangle-zero the diagonal 128-block [q0, q0+128)
                        # on the otherwise-idle gpsimd: keep where j >= p
                        nc.gpsimd.affine_select(
                            out=e[:, q0 : q0 + 128],
                            in_=e[:, q0 : q0 + 128],
                            compare_op=mybir.AluOpType.is_ge,
                            fill=0.0,
                            base=0,
                            channel_multiplier=-1,
                            pattern=[[1, 128]],
                        )
                    et.append(e)
                    # pre-sum E tiles on DVE (two tree levels) so the l
                    # matmuls stream 4x fewer columns through the PE
                    if kt % 2 == 1:
                        j = kt // 2
                        a, bt = et[2 * j], et[2 * j + 1]
                        s = epool.tile([128, QBLK], bf, tag=f"es{j}", name=f"es{j}")
                        if diag:
                            qa, qb = 256 * j, 256 * j + 128
                            # [qa, qb): only et[2j] is valid; [qb,..): both
                            nc.vector.tensor_copy(s[:, qa:qb], a[:, qa:qb])
                            nc.vector.tensor_add(s[:, qb:], a[:, qb:], bt[:, qb:])
                        else:
                            nc.vector.tensor_add(s[:], a[:], bt[:])
                        es1.append(s)
                    if kt % 4 == 3 and not diag:
                        j = kt // 4
                        a, bt = es1[2 * j], es1[2 * j + 1]
                        s = epool.tile(
                            [128, QBLK], bf, tag=f"es2_{j}", name=f"es2_{j}"
                        )
                        if diag:
                            qa, qb = 512 * j, 512 * j + 256
                            nc.vector.tensor_copy(s[:, qa:qb], a[:, qa:qb])
                            nc.vector.tensor_add(s[:, qb:], a[:, qb:], bt[:, qb:])
                        else:
                            nc.vector.tensor_add(s[:], a[:], bt[:])
                        es.append(s)

                lb = opool.tile([1, QBLK], f32, tag="lb")

                def emit_chunk(c, diag=diag, v_s=v_s, lb=lb, et=et, es=es, es1=es1, p=p):
                    sl0, sl1 = c * 512, (c + 1) * 512
                    kts = [
                        kt
                        for kt in range(NKT)
                        if not diag or 128 * kt < sl1
                    ]
                    po = ps_o.tile([128, 512], f32, tag="po")
                    for i, kt in enumerate(kts):
                        r0 = max(sl0, 128 * kt) if diag else sl0
                        nc.tensor.matmul(
                            po[:, r0 - sl0 : 512],
                            lhsT=v_s[:, kt * 128 : (kt + 1) * 128],
                            rhs=et[kt][:, r0:sl1],
                            start=(i == 0),
                            stop=(i == len(kts) - 1),
                            skip_group_check=diag,
                        )
                    pl = ps_l.tile([1, 512], f32, tag="pl")
                    if diag:
                        src, step = es1, 256
                    else:
                        src, step = es, 512
                    js = [j for j in range(len(src)) if not diag or step * j < sl1]
                    for i, j in enumerate(js):
                        r0 = max(sl0, step * j) if diag else sl0
                        nc.tensor.matmul(
                            pl[:, r0 - sl0 : 512],
                            lhsT=ones[:],
                            rhs=src[j][:, r0:sl1],
                            start=(i == 0),
                            stop=(i == len(js) - 1),
                            skip_group_check=diag,
                        )
                    ob = opool.tile([128, 512], f32, tag="ob")
                    nc.vector.tensor_copy(ob[:], po[:])
                    nc.sync.dma_start(accT[p][:, sl0:sl1], ob[:])
                    nc.vector.tensor_copy(lb[:, sl0:sl1], pl[:])

                if diag:
                    # chunk 0 only needs key tiles 0..3 -- emit its AV/l as
                    # soon as their exps are done, before scoring kts 4..7
                    for kt in range(4):
                        emit_kt(kt)
                    emit_chunk(0)
                    for kt in range(4, NKT):
                        emit_kt(kt)
                    emit_chunk(1)
                else:
                    for kt in range(NKT):
                        emit_kt(kt)
                    emit_chunk(0)
                    emit_chunk(1)
                nc.sync.dma_start(lsum[p], lb[:])
    return _patch_bass(nc)


_NC_CACHE = {}


def _get_nc(name):
    if name not in _NC_CACHE:
        _NC_CACHE[name] = build_qkv_nc() if name == "qkv" else build_attn_nc()
    return _NC_CACHE[name]


def _phase1_inmaps(x, W_qkv):
    xf = np.ascontiguousarray(x, dtype=np.float32).reshape(ROWS, D)
    W_qkv = np.asarray(W_qkv, dtype=np.float32)
    # wcat[p, o*D + i*128 + e] = W_qkv[o*128 + e, i*128 + p]  (o-major)
    w3 = W_qkv.astype(BF16).reshape(3, 128, NDT, 128)  # [o, e, i, p]
    wcat = np.ascontiguousarray(w3.transpose(3, 0, 2, 1).reshape(128, 3 * D))
    return [
        {
            "xT": np.ascontiguousarray(xf[c * RPC : (c + 1) * RPC].T.astype(BF16)),
            "wcat": wcat,
        }
        for c in range(N_CORES)
    ]


def _run_phase1(x, W_qkv):
    res1 = run_bass_kernel_spmd(
        _get_nc("qkv"), _phase1_inmaps(x, W_qkv), core_ids=list(range(N_CORES))
    )
    qTg = np.concatenate([res1.results[c]["qT"] for c in range(N_CORES)], axis=1)
    kTg = np.concatenate([res1.results[c]["kT"] for c in range(N_CORES)], axis=1)
    vTg = np.concatenate([res1.results[c]["vT"] for c in range(N_CORES)], axis=1)
    return qTg, kTg, vTg


def _blkT(g, b, j):  # [DA, QBLK] block j of batch b from transposed global
    s0 = b * S + j * QBLK
    return g[:, s0 : s0 + QBLK]


def _phase2_inmaps(qTg, kTg, vTg):
    in2 = []
    for c in range(N_CORES):
        b, h = divmod(c, 2)
        pairs = PAIRS_H0 if h == 0 else PAIRS_H1
        qkv = np.empty((NPAIR, 3, 128, QBLK), dtype=BF16)
        for p, (qb, kb) in enumerate(pairs):
            qkv[p, 0] = _blkT(qTg, b, qb)
            qkv[p, 1] = _blkT(kTg, b, kb)
            # v[kk, kt*128 + d] = V[kt*128 + kk, d]; V block = (vT block).T
            qkv[p, 2] = (
                _blkT(vTg, b, kb)
                .T.reshape(NKT2, 128, DA)
                .transpose(1, 0, 2)
                .reshape(128, QBLK)
            )
        in2.append({"qkv": qkv})
    return in2


def kernel(x, W_qkv):
    qTg, kTg, vTg = _run_phase1(x, W_qkv)
    res2 = run_bass_kernel_spmd(
        _get_nc("attn"), _phase2_inmaps(qTg, kTg, vTg), core_ids=list(range(N_CORES))
    )

    # ---- host combine ----
    out = np.empty((B, S, DA), dtype=np.float32)
    for c in range(N_CORES):
        b, h = divmod(c, 2)
        pairs = PAIRS_H0 if h == 0 else PAIRS_H1
        accT = res2.results[c]["accT"]  # [NPAIR, DA, QBLK] f32
        lsum = res2.results[c]["lsum"]  # [NPAIR, QBLK] f32
        for qb in set(q for q, _ in pairs):
            idx = [i for i, (q, _) in enumerate(pairs) if q == qb]
            acc = accT[idx].sum(axis=0)  # [DA, QBLK]
            l = lsum[idx].sum(axis=0)  # [QBLK]
            out[b, qb * QBLK : (qb + 1) * QBLK, :] = (acc / l).T
    return out



# revision 51
# speedup vs baseline: 1.1265x; 1.1265x over previous
"""Causal single-head attention (B=4, S=4096, D=2048, d_att=128) on 8 TRN2 cores.

Strategy (all shapes hardcoded; self-contained):
  Phase 1 (SPMD x8): fused QKV projection, data-parallel over the 16384
    flattened (b, s) rows -- 2048 rows per core. Split-fp8 matmul: x and W
    ship as an e4m3 "hi" part plus an e5m2 residual "lo" part (e5m2's wider
    exponent keeps the small residuals out of denormals); the product is
    hi*hi + hi*lo + lo*hi (dropping lo*lo, ~1e-4 relative) computed with
    fp8 DoubleRow matmuls that contract two 128-deep tiles per pass at 0.5
    cycles/column -- 1.33x the bf16 FLOP rate at bf16-level accuracy
    (end-to-end rel err 7.5e-3 vs the fp32 reference, verified on hw).
    The phase is jointly PE/DMA-bound (all DMA transfers serialize on the
    DMA engines): x streams in variable-width chunks (small head chunks so
    compute starts early, a small tail chunk so the final copy+DMA chain is
    short), weights split per output block so o=0 compute gates on a
    minimal prefix of the stream.
  Host: reshuffles Q/K/V into 5 balanced (q-block, k-block) "pairs" per core.
    Core (b, 0) handles query blocks {0, 3} of batch b, core (b, 1) blocks
    {1, 2} (blocks of 1024). Expanding each causal row-block into its
    k-prefix gives exactly 5 block-pairs per core: [diag, full, full, full,
    diag], identical structure on every core -> one SPMD program.
  Phase 2 (SPMD x8): ACT-bound (exp streams 1 elem/cycle/partition, and
    PSUM's 8 banks cap the score slots at 2, so exp granularity is one
    [128, 1024] key tile). Per key tile: scores S^T = K Q^T (PE, bf16) ->
    E = exp(scale*S^T) (ACT, unnormalized; max|scaled score| ~ 9 so no
    row-max pass) -> O^T += V^T E^T (PE). The AV accumulation is deferred
    one tile and interleaved as [AV(kt-1).c0, scores(kt+1), AV(kt-1).c1]:
    the deferred AV is already un-gated, covering the ~240 ns slot-release
    window after each exp, and the next scores tile lands before the
    current exp retires -- so ACT runs gap-free at its 1038 ns cadence.
    Causal triangle zeroing of the diagonal 128-block on the otherwise-idle
    gpsimd. Softmax denominator l = 1^T E^T on PE over a 2-3 level DVE
    pre-sum tree of E (8x fewer PE columns). Partials (acc, l) combine on
    the host: out = sum(acc) / sum(l) per query row.
  Compute dtype bf16 for attention (fp8 scores/AV fail the 2e-2 rel-err
  budget: measured 4.2e-2 / 2.0e-2+ on the real data), fp32 PSUM.
"""

import numpy as np
import ml_dtypes
import orjson

import concourse.bass as bass
import concourse.tile as tile
import concourse.mybir as mybir
from concourse.bass_utils import run_bass_kernel_spmd

BF16 = ml_dtypes.bfloat16
E4M3 = ml_dtypes.float8_e4m3fn
E5M2 = ml_dtypes.float8_e5m2
N_CORES = 8
B, S, D, DA = 4, 4096, 2048, 128
ROWS = B * S  # 16384
RPC = ROWS // N_CORES  # 2048 rows per core in phase 1
QBLK = 1024  # query/key block size for pair decomposition
NKT2 = QBLK // 128  # key tiles per pair
NPAIR = 5
SCALE = 1.0 / np.sqrt(DA)

# (q_block, k_block) pairs per half; p0 and p4 are causal-diagonal on all cores
PAIRS_H0 = [(0, 0), (3, 0), (3, 1), (3, 2), (3, 3)]
PAIRS_H1 = [(1, 1), (1, 0), (2, 0), (2, 1), (2, 2)]
DIAG_PAIRS = (0, 4)


# ---------------------------------------------------------------------------
# Walrus workaround: this neuronxcc build rejects >1 sync-wait command per
# instruction ("Too many sync wait commands"). Excess on_wait entries are
# hoisted onto preceding same-engine NoOps -- semantically identical since
# each engine executes its queue in order.
# ---------------------------------------------------------------------------
def _fix_bir_json(bir: bytes, max_waits: int = 1) -> bytes:
    m = orjson.loads(bir)
    counter = [0]
    for fn in m.get("functions", []):
        for bb in fn.get("blocks", []):
            out = []
            for inst in bb.get("instructions", []):
                si = inst.get("sync_info")
                waits = (si or {}).get("on_wait") or []
                if len(waits) > max_waits:
                    excess, keep = waits[:-max_waits], waits[-max_waits:]
                    for i in range(0, len(excess), max_waits):
                        counter[0] += 1
                        out.append(
                            {
                                "engine": inst["engine"],
                                "ins": [],
                                "name": f"I-waitfix-{counter[0]}",
                                "opcode": "NoOp",
                                "outs": [],
                                "sync_info": {
                                    "on_update": [],
                                    "on_wait": excess[i : i + max_waits],
                                },
                            }
                        )
                    si["on_wait"] = keep
                out.append(inst)
            bb["instructions"] = out
    return orjson.dumps(m)


def _patch_bass(nc):
    orig = nc.to_json_bytes
    nc.to_json_bytes = lambda: _fix_bir_json(orig())
    return nc


# ---------------------------------------------------------------------------
# Phase 1: QKV projection, split-fp8.  Per core:
#   xhi [128, NCH, NDT, CH] e4m3:  xhi[p, c, i, r] = x_hi[row c*CH+r, d i*128+p]
#   xlo same shape, e5m2 residual
#   whi [128, 3, NJP, 2, 128] e4m3: whi[p, o, j, t, e] = W_hi[o*128+e, (2j+t)*128+p]
#   wlo same shape, e5m2 residual
#   -> qT/kT/vT [DA, RPC] bf16
# ---------------------------------------------------------------------------
NDT = D // 128  # 16 contraction tiles
NJP = NDT // 2  # 8 DoubleRow tile-pairs
# variable chunk widths: small chunks at the head (so compute starts while
# the first x DMAs land) and at the tail (so the last output DMA chain is
# short); 512 in the steady state where DMA comfortably outruns the PE
CHS = [256, 256, 384, 448, 448, 256]
assert sum(CHS) == RPC
CH_OFF = [sum(CHS[:i]) for i in range(len(CHS))]
QKV_WARMUP = 6


def build_qkv_nc():
    nc = bass.Bass(
        "TRN2", target_bir_lowering=False, debug=False, enable_partition_id=False
    )
    bf = mybir.dt.bfloat16
    f32 = mybir.dt.float32
    fp8h = mybir.dt.float8e4
    fp8l = mybir.dt.float8e5
    DR = mybir.MatmulPerfMode.DoubleRow
    # x parts are packed chunk-contiguously: chunk c occupies columns
    # [NDT*CH_OFF[c], NDT*(CH_OFF[c]+W)) laid out as [i, r] per partition
    xhi = nc.dram_tensor("xhi", [128, NDT * RPC], fp8h, kind="ExternalInput").ap()
    xlo = nc.dram_tensor("xlo", [128, NDT * RPC], fp8l, kind="ExternalInput").ap()
    whi = nc.dram_tensor("whi", [128, 3, NJP, 2, 128], fp8h, kind="ExternalInput").ap()
    wlo = nc.dram_tensor("wlo", [128, 3, NJP, 2, 128], fp8l, kind="ExternalInput").ap()
    # fused output [o, e, row]
    qkv3 = nc.dram_tensor("qkv3", [3, DA, RPC], bf, kind="ExternalOutput").ap()

    with tile.TileContext(nc) as tc:
        import contextlib

        with contextlib.ExitStack() as ctx:
            xpool = ctx.enter_context(tc.tile_pool(name="xp", bufs=1))
            wpool = ctx.enter_context(tc.tile_pool(name="wp", bufs=1))
            opool = ctx.enter_context(tc.tile_pool(name="op", bufs=2))
            pspool = ctx.enter_context(tc.tile_pool(name="ps", bufs=2, space="PSUM"))

            whs = wpool.tile([128, 3, NJP, 2, 128], fp8h)
            wls = wpool.tile([128, 3, NJP, 2, 128], fp8l)

            def load_xc(c):
                # hi on the SP queue, lo on the ACT queue
                W = CHS[c]
                o0 = NDT * CH_OFF[c]
                o1 = o0 + NDT * W
                xh = xpool.tile([128, NDT, W], fp8h, tag=f"xh{c % 3}", name=f"xh{c}")
                xl = xpool.tile([128, NDT, W], fp8l, tag=f"xl{c % 3}", name=f"xl{c}")
                nc.sync.dma_start(xh[:], xhi[:, o0:o1])
                nc.scalar.dma_start(xl[:], xlo[:, o0:o1])
                return xh, xl

            # chunk 0's hi part leads; o=0 weights next (those two gate
            # the first term-group); everything else staggers behind
            xh0 = xpool.tile([128, NDT, CHS[0]], fp8h, tag="xh0", name="xh0")
            nc.sync.dma_start(xh0[:], xhi[:, 0 : NDT * CHS[0]])
            nc.sync.dma_start(whs[:, 0], whi[:, 0])
            xl0 = xpool.tile([128, NDT, CHS[0]], fp8l, tag="xl0", name="xl0")
            nc.scalar.dma_start(wls[:, 0], wlo[:, 0])
            nc.scalar.dma_start(xl0[:], xlo[:, 0 : NDT * CHS[0]])
            xcs = [(xh0, xl0), load_xc(1)]
            nc.sync.dma_start(whs[:, 1], whi[:, 1])
            nc.scalar.dma_start(wls[:, 1], wlo[:, 1])
            xcs.append(load_xc(2))
            nc.sync.dma_start(whs[:, 2], whi[:, 2])
            nc.scalar.dma_start(wls[:, 2], wlo[:, 2])

            # Warm up the PE (HAM clock ramp) on dummy data during the
            # initial DMA window so the real matmuls start at full clock.
            dummy = wpool.tile([128, 512], bf, tag="dummy")
            nc.vector.memset(dummy[:], 1.0)
            wps = pspool.tile([128, 512], f32, tag="ps0", name="warm")
            for _ in range(QKV_WARMUP):
                nc.tensor.matmul(
                    wps[:],
                    lhsT=dummy[:, 0:128],
                    rhs=dummy[:],
                    start=True,
                    stop=True,
                )

            NC_ = len(CHS)
            for c in range(NC_):
                if c + 3 < NC_:
                    xcs.append(load_xc(c + 3))
                xh, xl = xcs[c]
                W = CHS[c]
                sl = slice(CH_OFF[c], CH_OFF[c] + W)
                last = c == NC_ - 1
                ps = [
                    pspool.tile([128, 512], f32, tag=f"ps{o}", name=f"ps_{c}_{o}")
                    for o in range(3)
                ]
                # the three split-fp8 term-groups; x_hi terms first so the
                # x_lo DMA has a full two groups of slack
                terms = [(whs, xh), (wls, xh), (whs, xl)]

                def emit_group(o, g):
                    w, x = terms[g]
                    for j in range(NJP):
                        nc.tensor.matmul(
                            ps[o][:, 0:W],
                            lhsT=w[:, o, j],
                            rhs=x[:, 2 * j : 2 * j + 2, :],
                            start=(g == 0 and j == 0),
                            stop=(g == 2 and j == NJP - 1),
                            perf_mode=DR,
                            skip_group_check=True,
                        )

                def emit_out(o, ob):
                    nc.scalar.copy(ob[:, o, :], ps[o][:, 0:W])

                ob = opool.tile([128, 3, W], bf, tag="ob", name=f"ob{c}")
                if last:
                    # close each o-group serially so output DMAs drain
                    # while the remaining groups still compute
                    for o in range(3):
                        for g in range(3):
                            emit_group(o, g)
                        emit_out(o, ob)
                        eng = nc.scalar if o == 2 else nc.sync
                        eng.dma_start(qkv3[o][:, sl], ob[:, o, :])
                else:
                    # interleave the o-groups per term so the lo-term
                    # operands have maximal DMA slack
                    for g in range(3):
                        for o in range(3):
                            emit_group(o, g)
                    for o in range(3):
                        emit_out(o, ob)
                    nc.sync.dma_start(
                        qkv3[:, :, sl].rearrange("t p r -> p t r"), ob[:]
                    )
    return _patch_bass(nc)


# ---------------------------------------------------------------------------
# Phase 2: pair attention. Per core:
#   qkv[p, 0] = Q^T block [128, QBLK], [p, 1] = K^T block, [p, 2] = V block
#   pre-permuted (v[kk, kt*128 + d] = V[kt*128 + kk, d]) so that
#   v[:, kt*128:(kt+1)*128] is the [k, d] tile for the AV lhsT.
#   -> accT [NPAIR, DA, QBLK] f32 (unnormalized O^T), lsum [NPAIR, QBLK] f32
# ---------------------------------------------------------------------------
ATT_WARMUP = 7


def build_attn_nc():
    nc = bass.Bass(
        "TRN2", target_bir_lowering=False, debug=False, enable_partition_id=False
    )
    bf = mybir.dt.bfloat16
    f32 = mybir.dt.float32
    qkv = nc.dram_tensor("qkv", [NPAIR, 3, 128, QBLK], bf, kind="ExternalInput").ap()
    accT = nc.dram_tensor("accT", [NPAIR, DA, QBLK], f32, kind="ExternalOutput").ap()
    lsum = nc.dram_tensor("lsum", [NPAIR, QBLK], f32, kind="ExternalOutput").ap()

    NKT = QBLK // 128  # 8 key tiles per pair
    Exp = mybir.ActivationFunctionType.Exp

    with tile.TileContext(nc) as tc:
        import contextlib

        with contextlib.ExitStack() as ctx:
            const = ctx.enter_context(tc.tile_pool(name="const", bufs=1))
            inp = ctx.enter_context(tc.tile_pool(name="inp", bufs=2))
            epool = ctx.enter_context(tc.tile_pool(name="ep", bufs=2))
            opool = ctx.enter_context(tc.tile_pool(name="op", bufs=3))
            # PSUM budget (8 banks): scores 2 slots x 2 banks, po 3 x 1, pl 1
            ps_s = ctx.enter_context(tc.tile_pool(name="pss", bufs=1, space="PSUM"))
            ps_o = ctx.enter_context(tc.tile_pool(name="pso", bufs=1, space="PSUM"))
            ps_l = ctx.enter_context(tc.tile_pool(name="psl", bufs=1, space="PSUM"))

            ones = const.tile([128, 1], bf)
            nc.vector.memset(ones[:], 1.0)

            def load_pair(p):
                qkv_s = inp.tile([128, 3, QBLK], bf, tag="qkv", name=f"qkv{p}")
                if p == 0:
                    # split so the scores-gating q+k lands ~0.8us sooner
                    nc.sync.dma_start(
                        qkv_s[:, 0:2, :], qkv[p, 0:2].rearrange("t p q -> p t q")
                    )
                    nc.scalar.dma_start(qkv_s[:, 2, :], qkv[p, 2])
                else:
                    nc.sync.dma_start(
                        qkv_s[:], qkv[p].rearrange("t p q -> p t q")
                    )
                return qkv_s

            pair_in = [load_pair(0), load_pair(1)]

            # PE warm-up on dummy data during the first input DMA
            dummy = const.tile([128, 512], bf)
            nc.vector.memset(dummy[:], 1.0)
            def emit_warmup(n, tag="po0"):
                wt = ps_o.tile([128, 512], f32, tag=tag, name=f"warm_{tag}_{n}")
                for _ in range(n):
                    nc.tensor.matmul(
                        wt[:],
                        lhsT=dummy[:, 0:128],
                        rhs=dummy[:],
                        start=True,
                        stop=True,
                    )

            emit_warmup(ATT_WARMUP)

            # Per-pair state built by emit_kt, consumed by emit_end.
            pair_state = {}

            def emit_exp(p, kt):
                st = pair_state[p]
                diag = st["diag"]
                et = st["et"]
                q0 = 128 * kt if diag else 0  # causal: keys kt*128.. only
                pss = st["pss"].pop(kt)
                e = epool.tile([128, QBLK], bf, tag=f"et{kt}", name=f"et_{p}_{kt}")
                nc.scalar.activation(e[:, q0:QBLK], pss[:, q0:QBLK], Exp, scale=SCALE)
                if diag:
                    # triangle-zero the diagonal 128-block [q0, q0+128) on
                    # the otherwise-idle gpsimd: keep where j >= p
                    nc.gpsimd.affine_select(
                        out=e[:, q0 : q0 + 128],
                        in_=e[:, q0 : q0 + 128],
                        compare_op=mybir.AluOpType.is_ge,
                        fill=0.0,
                        base=0,
                        channel_multiplier=-1,
                        pattern=[[1, 128]],
                    )
                et.append(e)

            def emit_av_c0(p, kt):
                st = pair_state[p]
                if kt in st["av_sched"][0]:
                    emit_av(p, kt, 0, st)

            def emit_av_c1_tree(p, kt):
                st = pair_state[p]
                diag = st["diag"]
                et, es1, es = st["et"], st["es1"], st["es"]
                # Chunk 1's group starts at kt==2 (kt 0/1 fold back in at kt
                # 4/5) so its po buffer stays readable for the previous
                # pair's emit_out.
                if kt in st["av_sched"][1] and kt >= 2:
                    emit_av(p, kt, 1, st)
                f0, f1 = (3, 4) if diag else (4, 5)
                if kt == f0:
                    emit_av(p, 0, 1, st)
                elif kt == f1:
                    emit_av(p, 1, 1, st)
                # pre-sum E tiles on DVE (tree levels) so the l matmuls
                # stream far fewer columns through the PE
                if kt % 2 == 1:
                    j = kt // 2
                    a, bt = et[2 * j], et[2 * j + 1]
                    s = epool.tile([128, QBLK], bf, tag=f"es{j}", name=f"es_{p}_{j}")
                    if diag:
                        qa, qb = 256 * j, 256 * j + 128
                        # [qa, qb): only et[2j] is valid; [qb,..): both
                        nc.vector.tensor_copy(s[:, qa:qb], a[:, qa:qb])
                        nc.vector.tensor_add(s[:, qb:], a[:, qb:], bt[:, qb:])
                    else:
                        nc.vector.tensor_add(s[:], a[:], bt[:])
                    es1.append(s)
                if kt % 4 == 3:
                    j = kt // 4
                    a, bt = es1[2 * j], es1[2 * j + 1]
                    s = epool.tile(
                        [128, QBLK], bf, tag=f"es2_{j}", name=f"es2_{p}_{j}"
                    )
                    if diag:
                        # es1[2j+1] only holds keys from col 512j+256 on
                        qa, qb = 512 * j, 512 * j + 256
                        nc.vector.tensor_copy(s[:, qa:qb], a[:, qa:qb])
                        nc.vector.tensor_add(s[:, qb:], a[:, qb:], bt[:, qb:])
                    else:
                        nc.vector.tensor_add(s[:], a[:], bt[:])
                    es.append(s)
                if kt == NKT - 1 and not diag:
                    s = epool.tile([128, QBLK], bf, tag="es3", name=f"es3_{p}")
                    nc.vector.tensor_add(s[:], es[0][:], es[1][:])
                    st["es3"] = s

            def emit_scores(p, kt):
                st = pair_state[p]
                diag, qkv_s = st["diag"], st["qkv_s"]
                qt_s = qkv_s[:, 0, :]
                kt_s = qkv_s[:, 1, :]
                q0 = 128 * kt if diag else 0
                pss = ps_s.tile(
                    [128, QBLK], f32, tag=f"pss{kt % 2}", name=f"pss_{p}_{kt}"
                )
                c0 = q0
                while c0 < QBLK:
                    c1 = min((c0 // 512 + 1) * 512, QBLK)
                    nc.tensor.matmul(
                        pss[:, c0:c1],
                        lhsT=kt_s[:, kt * 128 : (kt + 1) * 128],
                        rhs=qt_s[:, c0:c1],
                        start=True,
                        stop=True,
                    )
                    c0 = c1
                return pss

            def emit_av(p, kt, c, st):
                diag, qkv_s = st["diag"], st["qkv_s"]
                v_s = qkv_s[:, 2, :]
                q0 = 128 * kt if diag else 0
                sl0, sl1 = 512 * c, 512 * (c + 1)
                r0 = max(sl0, q0)
                sched = st["av_sched"][c]
                nc.tensor.matmul(
                    st["po"][c][:, r0 - sl0 : 512],
                    lhsT=v_s[:, kt * 128 : (kt + 1) * 128],
                    rhs=st["et"][kt][:, r0:sl1],
                    start=(kt == sched[0]),
                    stop=(kt == sched[-1]),
                    skip_group_check=True,
                )

            def start_pair(p):
                diag = p in DIAG_PAIRS
                if p + 2 < NPAIR:
                    pair_in.append(load_pair(p + 2))
                po = [
                    ps_o.tile(
                        [128, 512], f32,
                        tag=f"po{(2 * p + c) % 3}", name=f"po_{p}_{c}",
                    )
                    for c in range(2)
                ]
                av0 = [kt for kt in range(NKT) if not diag or 128 * kt < 512]
                # chunk 1: group starts at kt 2 (so its po buffer stays
                # readable for the previous pair's emit_out, emitted between
                # kt 1 and kt 2); kt 0/1 fold back in during the ACT-bound
                # stretch at kt 4/5
                av1 = ([2, 3, 0, 4, 1] + list(range(5, NKT))) if diag else (
                    [2, 3, 4, 0, 5, 1, 6, 7])
                pair_state[p] = {
                    "diag": diag,
                    "qkv_s": pair_in[p],
                    "po": po,
                    "et": [],
                    "es1": [],
                    "es": [],
                    "es3": None,
                    "pss": {},
                    "av_sched": [av0, av1],
                }

            def emit_out_chunk(p, c, eng=None):
                # po -> SBUF -> HBM; depends only on the AV group close, not
                # on the DVE pre-sum tree, so it can run right at pair end.
                # The copy runs on the mostly-idle gpsimd (DVE owns the
                # pre-sum tree; ACT owns exp); the final pair uses ACT,
                # which is idle once the last exp retires.
                st = pair_state[p]
                sl0, sl1 = 512 * c, 512 * (c + 1)
                ob = opool.tile([128, 512], f32, tag=f"ob{c}", name=f"ob_{p}_{c}")
                if eng == "act":
                    nc.scalar.copy(ob[:], st["po"][c][:])
                else:
                    nc.vector.tensor_copy(ob[:], st["po"][c][:])
                nc.sync.dma_start(accT[p][:, sl0:sl1], ob[:])

            def emit_l_chunk(p, c, final=False):
                # l = 1^T E^T over the pre-summed tree: full pairs read the
                # single level-3 tile, diag pairs the valid level-2 tiles
                st = pair_state[p]
                diag, es = st["diag"], st["es"]
                sl0, sl1 = 512 * c, 512 * (c + 1)
                pl = ps_l.tile([1, 512], f32, tag="pl", name=f"pl_{p}_{c}")
                srcs = [st["es3"]] if not diag else [
                    es[j] for j in range(2) if 512 * j < sl1
                ]
                for i, s in enumerate(srcs):
                    nc.tensor.matmul(
                        pl[:],
                        lhsT=ones[:],
                        rhs=s[:, sl0:sl1],
                        start=(i == 0),
                        stop=(i == len(srcs) - 1),
                        skip_group_check=True,
                    )
                lbc = opool.tile([1, 512], f32, tag="lbc", name=f"lbc_{p}_{c}")
                nc.vector.tensor_copy(lbc[:], pl[:])
                nc.scalar.dma_start(lsum[p, sl0:sl1], lbc[:])

            # Software pipeline across pairs: pair p's end-block (l matmuls,
            # copies, output DMA) is emitted between kt 3 and kt 4 of pair
            # p+1 so the next pair's scores/exp keep PE and ACT fed while
            # the DVE pre-sum tree for pair p drains.
            last = NPAIR - 1
            start_pair(0)
            pair_state[0]["pss"][0] = emit_scores(0, 0)
            # (p, kt) of the AV/tree block deferred one exp behind
            pending = None
            for p in range(NPAIR):
                st = pair_state[p]
                if p == 0:
                    # fill the pipeline-fill bubble (PE has no deferred AV
                    # yet) with dummy matmuls into the po1 buffer -- safe,
                    # chunk 1's group only starts (and zeroes it) at kt 2
                    emit_warmup(2, tag="po1")
                for kt in range(NKT):
                    emit_exp(p, kt)
                    # one ungated deferred-AV matmul covers the ~240ns slot
                    # release window, then the next scores tile, then the
                    # rest of the deferred block -- ACT never waits
                    if pending is not None:
                        emit_av_c0(*pending)
                    if kt + 1 < NKT:
                        st["pss"][kt + 1] = emit_scores(p, kt + 1)
                    elif p + 1 < NPAIR:
                        if p + 1 not in pair_state:
                            start_pair(p + 1)
                        pair_state[p + 1]["pss"][0] = emit_scores(p + 1, 0)
                    if pending is not None:
                        emit_av_c1_tree(*pending)
                    pending = (p, kt)
                    # previous pair's output drains: diag pairs front-load
                    # them into the early blocks (their exp blocks shrink
                    # toward the end, leaving no PE slack there)
                    ko, kl = (0, 2) if st["diag"] else (2, 4)
                    if kt == ko and p > 0:
                        emit_out_chunk(p - 1, 0)
                        emit_out_chunk(p - 1, 1)
                    if kt == kl and p > 0:
                        emit_l_chunk(p - 1, 0)
                        emit_l_chunk(p - 1, 1)
                    if p == last and kt == 6:
                        # last pair is diagonal: its chunk-0 AV group and l
                        # tree close at kt 3 -- drain it early
                        emit_out_chunk(p, 0, eng="act")
                        emit_l_chunk(p, 0)
            emit_av_c0(*pending)
            emit_av_c1_tree(*pending)
            emit_out_chunk(last, 1, eng="act")
            emit_l_chunk(last, 1)
    return _patch_bass(nc)


_NC_CACHE = {}


def _get_nc(name):
    if name not in _NC_CACHE:
        _NC_CACHE[name] = build_qkv_nc() if name == "qkv" else build_attn_nc()
    return _NC_CACHE[name]


def _pack_x(rows_f32):
    """[RPC, D] f32 -> ([128, NDT*RPC] e4m3, same e5m2), chunk-contiguous."""
    hi = rows_f32.astype(E4M3)
    lo = (rows_f32 - hi.astype(np.float32)).astype(E5M2)

    def pack(a):
        # per chunk c of width W: [row r, d = i*128 + p] -> [p, i, r], all
        # chunks concatenated along the flat column axis
        parts = [
            np.ascontiguousarray(
                a[CH_OFF[c] : CH_OFF[c] + W].reshape(W, NDT, 128).transpose(2, 1, 0)
            ).reshape(128, NDT * W)
            for c, W in enumerate(CHS)
        ]
        return np.ascontiguousarray(np.concatenate(parts, axis=1))

    return pack(hi), pack(lo)


def _phase1_inmaps(x, W_qkv):
    xf = np.ascontiguousarray(x, dtype=np.float32).reshape(ROWS, D)
    Wf = np.asarray(W_qkv, dtype=np.float32)
    whi_f = Wf.astype(E4M3)
    wlo_f = (Wf - whi_f.astype(np.float32)).astype(E5M2)

    def pack_w(a):
        # [o*128+e, (2j+t)*128+p] -> [p, o, j, t, e]
        return np.ascontiguousarray(
            a.reshape(3, 128, NJP, 2, 128).transpose(4, 0, 2, 3, 1)
        )

    whi, wlo = pack_w(whi_f), pack_w(wlo_f)
    maps = []
    for c in range(N_CORES):
        xh, xl = _pack_x(xf[c * RPC : (c + 1) * RPC])
        maps.append({"xhi": xh, "xlo": xl, "whi": whi, "wlo": wlo})
    return maps


def _run_phase1(x, W_qkv):
    res1 = run_bass_kernel_spmd(
        _get_nc("qkv"), _phase1_inmaps(x, W_qkv), core_ids=list(range(N_CORES))
    )
    qTg = np.concatenate([res1.results[c]["qkv3"][0] for c in range(N_CORES)], axis=1)
    kTg = np.concatenate([res1.results[c]["qkv3"][1] for c in range(N_CORES)], axis=1)
    vTg = np.concatenate([res1.results[c]["qkv3"][2] for c in range(N_CORES)], axis=1)
    return qTg, kTg, vTg


def _blkT(g, b, j):  # [DA, QBLK] block j of batch b from transposed global
    s0 = b * S + j * QBLK
    return g[:, s0 : s0 + QBLK]


def _phase2_inmaps(qTg, kTg, vTg):
    in2 = []
    for c in range(N_CORES):
        b, h = divmod(c, 2)
        pairs = PAIRS_H0 if h == 0 else PAIRS_H1
        qkv = np.empty((NPAIR, 3, 128, QBLK), dtype=BF16)
        for p, (qb, kb) in enumerate(pairs):
            qkv[p, 0] = _blkT(qTg, b, qb)
            qkv[p, 1] = _blkT(kTg, b, kb)
            # v[kk, kt*128 + d] = V[kt*128 + kk, d]; V block = (vT block).T
            qkv[p, 2] = (
                _blkT(vTg, b, kb)
                .T.reshape(NKT2, 128, DA)
                .transpose(1, 0, 2)
                .reshape(128, QBLK)
            )
        in2.append({"qkv": qkv})
    return in2


def kernel(x, W_qkv):
    qTg, kTg, vTg = _run_phase1(x, W_qkv)
    res2 = run_bass_kernel_spmd(
        _get_nc("attn"), _phase2_inmaps(qTg, kTg, vTg), core_ids=list(range(N_CORES))
    )

    # ---- host combine ----
    out = np.empty((B, S, DA), dtype=np.float32)
    for c in range(N_CORES):
        b, h = divmod(c, 2)
        pairs = PAIRS_H0 if h == 0 else PAIRS_H1
        accT = res2.results[c]["accT"]  # [NPAIR, DA, QBLK] f32
        lsum = res2.results[c]["lsum"]  # [NPAIR, QBLK] f32
        for qb in set(q for q, _ in pairs):
            idx = [i for i, (q, _) in enumerate(pairs) if q == qb]
            acc = accT[idx].sum(axis=0)  # [DA, QBLK]
            l = lsum[idx].sum(axis=0)  # [QBLK]
            out[b, qb * QBLK : (qb + 1) * QBLK, :] = (acc / l).T
    return out
